# revision 14
# baseline (speedup 1.0000x reference)
"""Multi-head attention (B=4, S=2048, D=1024, H=16, causal) on 8 TRN2 NeuronCores.

Sharding: core i handles batch i//2 and head-group i%2 (8 heads / 512 projection
columns). Each core computes a partial output projection over its 512 rows of Wo;
the host sums the two partials per batch and adds (bv @ Wo + bo). No device
collectives.

v2 dataflow (bf16 matmuls, fp32 softmax), all per core:
  - j-major attention: for each query block j (512 queries), all 8 heads attend;
    projection groups and the j-1 output-projection tiles are woven in as PE
    filler between score/AV chunks.
  - Causal staircase: for (h, j), k-tiles 0..4j run full-width (512 queries) in
    2-k-tile PSUM chunks; the last three diagonal k-tiles r=1..3 only cover the
    un-masked query windows (384/128/256 wide) packed into one 768-col PSUM
    chunk, skipping the 6 fully-masked 128x128 blocks per (h, j). One wide exp
    per chunk; the only element-level masking left is four [128,128] triangle
    multiplies per (h, j) against a single shared triangle tile.
  - No bias matmuls: bk cancels in softmax (per-query constant), bv commutes
    through softmax (rows sum to 1) and is added on host as bv @ Wo, bq rides
    the ACT-engine Q eviction as a per-partition bias.
  - Denominator rides the AV matmul as a 65th V column; normalization uses a
    DMA-reshaped reciprocal ([1,512] -> [128,4]) and a GPSIMD partition
    broadcast, as in v1.
"""

import sys

for _p in ("/opt/trn_rl_repo",):
    if _p not in sys.path:
        sys.path.insert(0, _p)

import numpy as np
import ml_dtypes

BF16 = ml_dtypes.bfloat16

B, S, D = 4, 2048, 1024
H, HD = 16, 64
HPC = H // 2          # heads per core: 8
DPC = D // 2          # projection cols per core: 512
NCORES = 8
SCALE = 1.0 / np.sqrt(np.float32(HD))

_compiled = None


def _build():
    import concourse.bacc as bacc
    import concourse.mybir as mybir
    import concourse.tile as tile

    f32 = mybir.dt.float32
    bf = mybir.dt.bfloat16
    Exp = mybir.ActivationFunctionType.Exp
    Copy = mybir.ActivationFunctionType.Copy
    Ident = mybir.ActivationFunctionType.Identity

    nc = bacc.Bacc("TRN2", target_bir_lowering=False, debug=False)

    xtq = nc.dram_tensor("xtq", [D, S], bf, kind="ExternalInput")
    xtk = nc.dram_tensor("xtk", [D, S], bf, kind="ExternalInput")
    xtv = nc.dram_tensor("xtv", [D, S], bf, kind="ExternalInput")
    wq = nc.dram_tensor("wq", [D, DPC], bf, kind="ExternalInput")
    wk = nc.dram_tensor("wk", [D, DPC], bf, kind="ExternalInput")
    wv = nc.dram_tensor("wv", [D, DPC], bf, kind="ExternalInput")
    wo = nc.dram_tensor("wo", [DPC, D], bf, kind="ExternalInput")
    bqd = nc.dram_tensor("bqd", [128, 4], f32, kind="ExternalInput")
    trid = nc.dram_tensor("trid", [128, 128], bf, kind="ExternalInput")
    y = nc.dram_tensor("y", [S, D], bf, kind="ExternalOutput")

    NKD = D // 128        # 8 contraction tiles for projections
    NST = S // 128        # 16 seq tiles
    NSB = S // 512        # 4 seq blocks (query blocks j)
    NHP = HPC // 2        # 4 head pairs / 128-wide col groups

    # staircase packing for diagonal k-tiles r=1..3: (packed col offset, width)
    STAIR = {1: (0, 384), 3: (384, 128), 2: (512, 256)}

    with tile.TileContext(nc) as tc:
        with (
            tc.tile_pool(name="consts", bufs=1) as consts,
            tc.tile_pool(name="wqp", bufs=NKD) as wqp,
            tc.tile_pool(name="wkp", bufs=NKD) as wkp,
            tc.tile_pool(name="wvp", bufs=NKD) as wvp,
            tc.tile_pool(name="wop", bufs=4) as wop,
            tc.tile_pool(name="xt", bufs=1) as xtp,
            tc.tile_pool(name="qt", bufs=NHP) as qtp,
            tc.tile_pool(name="kt", bufs=NHP) as ktp,
            tc.tile_pool(name="vp", bufs=NST) as vpool,
            tc.tile_pool(name="ex", bufs=3) as expool,
            tc.tile_pool(name="ot", bufs=NHP) as otp,
            tc.tile_pool(name="ys", bufs=2) as ysp,
            tc.tile_pool(name="rb", bufs=2) as rbp,
            tc.tile_pool(name="rc", bufs=2) as rcp,
            tc.tile_pool(name="ps", bufs=2, space="PSUM") as psp,
            tc.tile_pool(name="av", bufs=2, space="PSUM") as avp,
            tc.tile_pool(name="sc", bufs=2, space="PSUM") as scp,
        ):
            # constants
            trit = consts.tile([128, 128], bf, tag="tri")
            nc.sync.dma_start(trit[:], trid.ap()[:])
            bqt = consts.tile([128, 4], f32, tag="bqt")
            nc.sync.dma_start(bqt[:], bqd.ap()[:])

            # PE warmup: small junk matmuls while input DMAs land, so HAM ramps
            # toward full clock before the first real projection group
            warm = consts.tile([128, 128], bf, tag="warm")
            nc.gpsimd.memset(warm[:], 0.25)
            wps = psp.tile([128, 512], f32, name="wps", tag="ps")
            for _ in range(24):
                nc.tensor.matmul(wps[:, 0:128], warm[:], warm[:],
                                 start=True, stop=True)

            # ---- input DMAs, in need order, striped across four DMA queues
            # (a single queue tops out around 200 GB/s -- it was the startup
            # bottleneck). Slot-sharing loads (xtk sb1-3 reuse the xtv slots
            # and so carry semaphore waits) stay on the sync queue, where no
            # compute instructions sit behind them.
            dmaq = [nc.sync, nc.scalar, nc.gpsimd]
            qi = [0]

            def dma_in(dst, src_ap):
                dmaq[qi[0] % 3].dma_start(dst, src_ap)
                qi[0] += 1

            wvt = []
            for kd in range(NKD):
                w = wvp.tile([128, DPC], bf, name=f"wv{kd}", tag="wv")
                dma_in(w[:], wv.ap()[kd * 128:(kd + 1) * 128, :])
                wvt.append(w)

            def make_quarter(src_t, prefix, tagp, kd, sb, q=None):
                xt = xtp.tile([128, 512], bf, name=f"{prefix}{kd}_{sb}",
                              tag=f"{tagp}{kd}_{sb}", bufs=1)
                src = src_t.ap()[kd * 128:(kd + 1) * 128, sb * 512:(sb + 1) * 512]
                if q is None:
                    dma_in(xt[:], src)
                else:
                    q.dma_start(xt[:], src)
                return xt

            xtv_q = [[None] * NSB for _ in range(NKD)]
            xtq_q = [[None] * NSB for _ in range(NKD)]
            xtk_q = [[None] * NSB for _ in range(NKD)]
            # sb0 of everything first so V st0-3 / Q sb0 / K sb0 start early.
            # xtk sb0 gets its own slots (no sharing with xtv sb0) so the K
            # projections don't wait on the V groups' reads.
            for kd in range(NKD):
                xtv_q[kd][0] = make_quarter(xtv, "xv", "xvk", kd, 0)
            wqt, wkt = [], []
            for kd in range(NKD):
                w = wqp.tile([128, DPC], bf, name=f"wq{kd}", tag="wq")
                dma_in(w[:], wq.ap()[kd * 128:(kd + 1) * 128, :])
                wqt.append(w)
            for kd in range(NKD):
                xtq_q[kd][0] = make_quarter(xtq, "xq", "xq", kd, 0)
            for kd in range(NKD):
                w = wkp.tile([128, DPC], bf, name=f"wk{kd}", tag="wk")
                dma_in(w[:], wk.ap()[kd * 128:(kd + 1) * 128, :])
                wkt.append(w)
            for kd in range(NKD):
                xtk_q[kd][0] = make_quarter(xtk, "xk", "xk0", kd, 0)
            for sb in range(1, NSB):
                for kd in range(NKD):
                    xtv_q[kd][sb] = make_quarter(xtv, "xv", "xvk", kd, sb)
                for kd in range(NKD):
                    xtq_q[kd][sb] = make_quarter(xtq, "xq", "xq", kd, sb)
                for kd in range(NKD):
                    xtk_q[kd][sb] = make_quarter(xtk, "xk", "xvk", kd, sb,
                                                 q=nc.sync)
                if sb == 1:
                    wot = []
                    for hp in range(4):
                        w = wop.tile([128, D], bf, name=f"wo{hp}", tag="wo")
                        dma_in(w[:], wo.ap()[hp * 128:(hp + 1) * 128, :])
                        wot.append(w)

            # ---- V projection groups (natural layout, [8 heads x 65] incl.
            # ones column for the softmax denominator; no bias)
            vts = [vpool.tile([128, HPC * 65], bf, name=f"v{st}", tag="v")
                   for st in range(NST)]

            def v_group(st):
                def group():
                    ps = psp.tile([128, 512], f32, name="psv", tag="ps")
                    for kd in range(NKD):
                        nc.tensor.matmul(
                            ps[:],
                            xtv_q[kd][st // 4][:, (st % 4) * 128:(st % 4 + 1) * 128],
                            wvt[kd][:],
                            start=(kd == 0), stop=(kd == NKD - 1),
                        )
                    vt = vts[st]
                    v3 = vt[:].rearrange("p (h c) -> p h c", h=HPC, c=65)
                    nc.vector.tensor_copy(
                        v3[:, :, 0:64],
                        ps[:].rearrange("p (h c) -> p h c", h=HPC, c=64),
                    )
                    nc.gpsimd.memset(v3[:, :, 64:65], 1.0)
                return group

            # ---- QT / KT projection groups (transposed layout [cols, seq])
            qts, kts = [], []
            for pool, lst, nm in ((qtp, qts, "qt"), (ktp, kts, "kt")):
                for hp in range(NHP):
                    lst.append(pool.tile([128, S], bf, name=f"{nm}{hp}", tag=nm))

            def q_group(hp, sb):
                def group():
                    ps = psp.tile([128, 512], f32, name="psq", tag="ps")
                    for kd in range(NKD):
                        nc.tensor.matmul(
                            ps[:],
                            wqt[kd][:, hp * 128:(hp + 1) * 128],
                            xtq_q[kd][sb][:],
                            start=(kd == 0), stop=(kd == NKD - 1),
                        )
                    # bq rides the eviction as a per-partition ACT bias
                    nc.scalar.activation(
                        qts[hp][:, sb * 512:(sb + 1) * 512], ps[:], Ident,
                        bias=bqt[:, hp:hp + 1])
                return group

            def k_group(hp, sb):
                def group():
                    ps = psp.tile([128, 512], f32, name="psk", tag="ps")
                    for kd in range(NKD):
                        nc.tensor.matmul(
                            ps[:],
                            wkt[kd][:, hp * 128:(hp + 1) * 128],
                            xtk_q[kd][sb][:],
                            start=(kd == 0), stop=(kd == NKD - 1),
                        )
                    nc.vector.tensor_copy(kts[hp][:, sb * 512:(sb + 1) * 512], ps[:])
                return group

            ots = [otp.tile([128, S], bf, name=f"ot{i}", tag="ot") for i in range(NHP)]

            def yproj_group(st, eb):
                def group():
                    ps = psp.tile([128, 512], f32, name="psy", tag="ps")
                    for hp in range(NHP):
                        nc.tensor.matmul(
                            ps[:],
                            ots[hp][:, st * 128:(st + 1) * 128],
                            wot[hp][:, eb * 512:(eb + 1) * 512],
                            start=(hp == 0), stop=(hp == NHP - 1),
                        )
                    ys = ysp.tile([128, 512], bf, name="ys", tag="ys")
                    if (st + eb) % 2 == 0:
                        nc.vector.tensor_copy(ys[:], ps[:])
                    else:
                        nc.scalar.activation(ys[:], ps[:], Copy)
                    nc.sync.dma_start(
                        y.ap()[st * 128:(st + 1) * 128, eb * 512:(eb + 1) * 512],
                        ys[:],
                    )
                return group

            # ---- phase filler schedule (j-major attention). Within a phase,
            # fillers are ordered by DMA arrival: yproj (wo landed long ago)
            # first, then V/Q/K groups whose x quarters stream in during the
            # phase -- a popped filler whose input is still in flight stalls
            # the in-order PE queue.
            phase_fillers = {
                0: ([v_group(st) for st in range(4, 8)]
                    + [q_group(hp, 1) for hp in range(NHP)]
                    + [k_group(hp, 1) for hp in range(NHP)]),
                1: ([yproj_group(st, eb) for st in range(0, 4) for eb in range(2)]
                    + [v_group(st) for st in range(8, 12)]
                    + [q_group(hp, 2) for hp in range(NHP)]
                    + [k_group(hp, 2) for hp in range(NHP)]),
                2: ([yproj_group(st, eb) for st in range(4, 8) for eb in range(2)]
                    + [v_group(st) for st in range(12, 16)]
                    + [q_group(hp, 3) for hp in range(NHP)]
                    + [k_group(hp, 3) for hp in range(NHP)]),
                3: [yproj_group(st, eb) for st in range(8, 12) for eb in range(2)],
            }

            # ---- attention
            # pending[0] carries the delayed AV emit of the previous chunk --
            # including across attend/phase boundaries, so the PE never sits on
            # an exp tail: the next attend's scores interleave with it.
            pending = [None]

            def flush_pending():
                if pending[0] is not None:
                    pending[0]()
                    pending[0] = None

            def attend(h, j, pop):
                hp, sub = h // 2, h % 2
                base = sub * 64
                qt_h = qts[hp][base:base + 64, :]
                kt_h = kts[hp][base:base + 64, :]
                av = avp.tile([128, 512], f32, name="av", tag="av")

                # chunks: pairs of full-width k-tiles 0..4j-1, then k-tile 4j
                # (the r0 diagonal) alone, then the packed staircase r=1..3
                chunks = [list(range(c0, min(c0 + 2, 4 * j)))
                          for c0 in range(0, 4 * j, 2)]
                chunks.append([4 * j])       # r0, full width, triangle at col 0
                chunks.append("stair")

                first_av = [True]

                def make_av_full(ex, kts_c, stop):
                    def emit():
                        for r, kti in enumerate(kts_c):
                            nc.tensor.matmul(
                                av[0:65, :],
                                vts[kti][:, h * 65:(h + 1) * 65],
                                ex[:, r * 512:(r + 1) * 512],
                                start=(first_av[0] and r == 0),
                                stop=(stop and r == len(kts_c) - 1),
                            )
                        first_av[0] = False
                    return emit

                def make_av_stair(ex):
                    def emit():
                        last = list(STAIR)[-1]
                        for r, (off, wid) in STAIR.items():
                            nc.tensor.matmul(
                                av[0:65, 128 * r:512],
                                vts[4 * j + r][:, h * 65:(h + 1) * 65],
                                ex[:, off:off + wid],
                                start=False, stop=(r == last),
                            )
                        # normalize immediately after the closing AV:
                        # denom -> [128,4] reshape -> fast recip -> bcast
                        avs = ysp.tile([65, 512], f32, name="avs", tag="ys")
                        nc.vector.tensor_copy(avs[:], av[0:65, :])
                        rsh = rcp.tile([128, 4], f32, name="rsh", tag="rsh")
                        nc.gpsimd.dma_start(rsh[:], avs[64:65, :])
                        rr_t = rcp.tile([128, 4], f32, name="rr", tag="rr")
                        nc.vector.reciprocal(rr_t[:], rsh[:])
                        rrow = rcp.tile([1, 512], f32, name="rrow", tag="rrow")
                        nc.gpsimd.dma_start(rrow[:], rr_t[:])
                        rb = rbp.tile([64, 512], f32, name="rb", tag="rb")
                        nc.gpsimd.partition_broadcast(rb[:], rrow[:], channels=64)
                        nc.vector.tensor_mul(
                            ots[hp][base:base + 64, j * 512:(j + 1) * 512],
                            avs[0:64, :],
                            rb[:],
                        )
                    return emit

                for ci, ch in enumerate(chunks):
                    sc = scp.tile([128, 1024], f32, name="sc", tag="sc")
                    ex = expool.tile([128, 1024], bf, name="ex", tag="ex")
                    if ch == "stair":
                        for r, (off, wid) in STAIR.items():
                            nc.tensor.matmul(
                                sc[:, off:off + wid],
                                kt_h[:, (4 * j + r) * 128:(4 * j + r + 1) * 128],
                                qt_h[:, j * 512 + 128 * r:(j + 1) * 512],
                                start=True, stop=True,
                            )
                        nc.scalar.activation(ex[:, 0:768], sc[:, 0:768], Exp,
                                             scale=float(SCALE))
                        for r, (off, wid) in STAIR.items():
                            nc.vector.tensor_mul(
                                ex[:, off:off + 128],
                                ex[:, off:off + 128],
                                trit[:],
                            )
                        this_av = make_av_stair(ex)
                    else:
                        for r, kti in enumerate(ch):
                            nc.tensor.matmul(
                                sc[:, r * 512:(r + 1) * 512],
                                kt_h[:, kti * 128:(kti + 1) * 128],
                                qt_h[:, j * 512:(j + 1) * 512],
                                start=True, stop=True,
                            )
                        nw = len(ch) * 512
                        nc.scalar.activation(ex[:, 0:nw], sc[:, 0:nw], Exp,
                                             scale=float(SCALE))
                        if ch[-1] == 4 * j:   # r0 chunk: triangle at col 0
                            nc.vector.tensor_mul(
                                ex[:, (len(ch) - 1) * 512:(len(ch) - 1) * 512 + 128],
                                ex[:, (len(ch) - 1) * 512:(len(ch) - 1) * 512 + 128],
                                trit[:],
                            )
                        this_av = make_av_full(ex, ch, stop=False)
                    pop()
                    flush_pending()
                    pending[0] = this_av

            # upfront groups: V st0-3, Q sb0, K sb0
            for st in range(4):
                v_group(st)()
            for hp in range(NHP):
                q_group(hp, 0)()
            for hp in range(NHP):
                k_group(hp, 0)()

            for j in range(NSB):
                fillers = phase_fillers[j]
                nchunks = 8 * (j + 2)
                state = {"chunk": 0, "popped": 0}

                def pop():
                    state["chunk"] += 1
                    want = state["chunk"] * len(fillers) // nchunks
                    while state["popped"] < min(want, len(fillers)):
                        fillers[state["popped"]]()
                        state["popped"] += 1

                for h in range(HPC):
                    attend(h, j, pop)
                # drain remaining fillers; flush the last attend's AV +
                # normalize after the first one so its exp latency and the
                # normalize chain hide under filler matmuls
                if state["popped"] < len(fillers):
                    fillers[state["popped"]]()
                    state["popped"] += 1
                    flush_pending()
                while state["popped"] < len(fillers):
                    fillers[state["popped"]]()
                    state["popped"] += 1

            flush_pending()
            for st in range(12, 16):
                for eb in range(2):
                    yproj_group(st, eb)()

    nc.compile()
    return nc


def _tri_mask():
    # tri[k, q] = 1 iff key k <= query q within a 128x128 diagonal block
    return np.triu(np.ones((128, 128), np.float32)).astype(BF16)


def _shard_inputs(q_in, k_in, v_in, Wq, bq, Wk, bk, Wv, bv, Wo, bo):
    tri = _tri_mask()
    in_maps = []
    for core in range(NCORES):
        b, g = core // 2, core % 2
        cs = slice(g * DPC, (g + 1) * DPC)
        in_maps.append({
            "xtq": np.ascontiguousarray(q_in[b].T).astype(BF16),
            "xtk": np.ascontiguousarray(k_in[b].T).astype(BF16),
            "xtv": np.ascontiguousarray(v_in[b].T).astype(BF16),
            "wq": Wq[:, cs].astype(BF16),
            "wk": Wk[:, cs].astype(BF16),
            "wv": Wv[:, cs].astype(BF16),
            "wo": np.ascontiguousarray(Wo[cs, :]).astype(BF16),
            "bqd": np.ascontiguousarray(
                bq[cs].reshape(4, 128).T).astype(np.float32),
            "trid": tri,
        })
    return in_maps


def kernel(q_in, k_in, v_in, Wq, bq, Wk, bk, Wv, bv, Wo, bo, _trace=False):
    from concourse.bass_utils import run_bass_kernel_spmd

    global _compiled
    if _compiled is None:
        _compiled = _build()

    args = [np.asarray(a, np.float32) for a in
            (q_in, k_in, v_in, Wq, bq, Wk, bk, Wv, bv, Wo, bo)]
    in_maps = _shard_inputs(*args)
    res = run_bass_kernel_spmd(
        _compiled, in_maps, core_ids=list(range(NCORES)), trace=_trace,
    )
    # bk cancels in softmax; bv commutes through (rows sum to 1): fold on host
    tail = (args[8].astype(np.float32) @ args[9].astype(np.float32)
            + args[10].astype(np.float32))
    out = np.empty((B, S, D), np.float32)
    for b in range(B):
        out[b] = (res.results[2 * b]["y"].astype(np.float32)
                  + res.results[2 * b + 1]["y"].astype(np.float32) + tail)
    if _trace:
        kernel.last_results = res
    return out
